# revision 1
# baseline (speedup 1.0000x reference)
"""Trainium2 Bass kernel for nn_GRU_15461882266204 (minGRU with causal conv gate).

Math (reference):
  w0 = x @ w_w.T ; z0 = x @ wz_w.T ; th = x @ wh_w.T          (S,H)
  z  = sigmoid(causal_conv4(z0, conv_w, segment-masked))
  a  = (1-z) * (1-start) ; b = z * th
  h_t = a_t * h_{t-1} + b_t                                    (scan over S)
  out = (h * silu(w0)) @ wo_w.T                                (S,D)

Strategy: sequence-parallel over 8 NeuronCores (1024 positions each, all 5632
channels per core). Projections as fp32r matmuls with D on the contraction
partitions (x pre-transposed on host). Conv + gating elementwise on DVE with
host-precomputed boundary masks (fully data-driven, SPMD-identical program).
Scan via the hardware tensor_tensor_scan instruction (channels on partitions,
time on the free axis). Cross-core scan carry: each core computes a local scan
from 0 plus the in-chunk cumprod A; a 360KB AllGather of (A_end, h_end)
summaries lets every core compute the carry chain redundantly and fix up
h_true = h_local + A * carry. Down-projection in bf16; output is
sequence-sharded so the host just concatenates (no all-reduce).
"""
import sys

sys.path.insert(0, "/opt/trn_rl_repo")

import numpy as np

import concourse.bacc as bacc
import concourse.mybir as mybir
import concourse.tile as tile
from concourse.bass_utils import run_bass_kernel_spmd

try:
    import ml_dtypes

    BF16 = np.dtype(ml_dtypes.bfloat16)
except ImportError:  # pragma: no cover
    BF16 = None

F32 = mybir.dt.float32
F32R = mybir.dt.float32r
MBF16 = mybir.dt.bfloat16
AL = mybir.AluOpType
ACTF = mybir.ActivationFunctionType

P = 128
CONV = 4
# matmul input dtype: bf16 enables the fast weight load path (LDWEIGHTS fully
# hidden behind the matmul stream); fp32r is ~1.3x slower but more accurate.
MM_BF16 = True
# 3 history columns are needed for the conv taps; we pad to 4 (one dead
# leading column) because fp32r matmuls require an even moving free-dim.
HIST = 4


def _ntiles(total, maxn=512):
    """Chop `total` into pieces of at most maxn: [(offset, size), ...]."""
    out = []
    o = 0
    while o < total:
        n = min(maxn, total - o)
        out.append((o, n))
        o += n
    return out


def build_gru_kernel(D, H, SC, NC, phases="ABCD"):
    """Build the SPMD per-core program. SC = sequence chunk per core."""
    KT = D // P    # contraction k-tiles
    MT = H // P    # hidden m-tiles
    SCH = SC + HIST
    z_nt = _ntiles(SCH)        # n-tiles for z_pre (includes 3 history cols)
    s_nt = _ntiles(SC)         # n-tiles for ht / w0
    MPT = SC // P              # output row tiles (s on partitions)
    NPT_D = _ntiles(D)         # output col tiles of 512
    # down-proj blocking: a m'-tiles x b n'-tiles concurrently, a*b <= 8 psum banks
    a_blk = min(MPT, 4)
    b_blk = min(len(NPT_D), 2)

    nc = bacc.Bacc(None, target_bir_lowering=False, debug=False)

    mmdt = MBF16 if MM_BF16 else F32
    xt_in = nc.declare_dram_parameter("xt", [P, KT, SCH], mmdt, isOutput=False)
    wz_in = nc.declare_dram_parameter("wz", [MT, P, KT, P], mmdt, isOutput=False)
    wh_in = nc.declare_dram_parameter("wh", [MT, P, KT, P], mmdt, isOutput=False)
    w_in = nc.declare_dram_parameter("w", [MT, P, KT, P], mmdt, isOutput=False)
    wo_in = nc.declare_dram_parameter("wo", [MT, P, D], MBF16, isOutput=False)
    cw_in = nc.declare_dram_parameter("cw", [MT, P, CONV], F32, isOutput=False)
    u_in = nc.declare_dram_parameter("u", [P, SC + 2], F32, isOutput=False)
    sel_in = nc.declare_dram_parameter("sel", [P, NC], F32, isOutput=False)
    out_d = nc.declare_dram_parameter("out", [SC, D], F32, isOutput=True)
    dbg_out = None
    if phases != "ABCD":
        # debug dumps: h_loc, A (phase A) / mycarry / g
        dbg_out = nc.declare_dram_parameter(
            "dbg", [3, MT, P, SC], F32, isOutput=True
        )

    with tile.TileContext(nc) as tc:
        with (
            tc.tile_pool(name="const", bufs=1) as cpool,
            tc.tile_pool(name="wts", bufs=2) as wpool,
            tc.tile_pool(name="work", bufs=2) as wk,
            tc.tile_pool(name="psum", bufs=8, space="PSUM") as pp,
            tc.tile_pool(name="dram", bufs=1, space="DRAM") as dp,
        ):
            # ---- resident tiles -------------------------------------------------
            sbdt = MBF16 if MM_BF16 else F32R
            xt_sb = cpool.tile([P, KT, SCH], sbdt, tag="xt")
            nc.sync.dma_start(xt_sb[:], xt_in[:])
            u_sb = cpool.tile([P, SC + 2], F32, tag="u")
            nc.sync.dma_start(u_sb[:], u_in[:])
            sel_sb = cpool.tile([P, NC], F32, tag="sel")
            nc.sync.dma_start(sel_sb[:], sel_in[:])
            ones = cpool.tile([P, SC], F32, tag="ones")
            nc.any.memset(ones[:], 1.0)
            summA = cpool.tile([P, MT], F32, tag="summA")
            summH = cpool.tile([P, MT], F32, tag="summH")

            # internal DRAM bounce buffers
            hl_d = dp.tile([MT, P, SC], F32)
            A_d = dp.tile([MT, P, SC], F32)
            g_d = dp.tile([MT, P, SC], MBF16)
            summ_d = dp.tile([P, 2 * MT], F32)
            gath_d = dp.tile([NC, P, 2 * MT], F32, addr_space="Shared")

            # ---- phase A: z/ht matmuls, conv, gating, local scans ---------------
            scopeA = nc.named_scope("phaseA"); scopeA.__enter__()
            for m in range(MT):
                cw_sb = wk.tile([P, CONV], F32, tag="cw")
                nc.sync.dma_start(cw_sb[:], cw_in[m])
                wz_sb = wpool.tile([P, KT, P], sbdt, tag="wz")
                nc.sync.dma_start(wz_sb[:], wz_in[m])
                wh_sb = wpool.tile([P, KT, P], sbdt, tag="wh")
                nc.sync.dma_start(wh_sb[:], wh_in[m])

                # z_pre = wz_m.T @ x over SC+3 cols (3 history cols included)
                z_pre = wk.tile([P, SCH], F32, tag="zpre")
                for (no, nn) in z_nt:
                    ps = pp.tile([P, 512], F32, tag="ps")
                    for k in range(KT):
                        nc.tensor.matmul(
                            ps[:, :nn],
                            wz_sb[:, k, :],
                            xt_sb[:, k, no : no + nn],
                            start=(k == 0),
                            stop=(k == KT - 1),
                        )
                    nc.scalar.copy(z_pre[:, no : no + nn], ps[:, :nn])

                # ht matmuls (positions [0, SC) = cols [3, SCH))
                ps_h = []
                for (no, nn) in s_nt:
                    ps = pp.tile([P, 512], F32, tag="ps")
                    for k in range(KT):
                        nc.tensor.matmul(
                            ps[:, :nn],
                            wh_sb[:, k, :],
                            xt_sb[:, k, HIST + no : HIST + no + nn],
                            start=(k == 0),
                            stop=(k == KT - 1),
                        )
                    ps_h.append((no, nn, ps))

                # masked shifted taps: yk(t) = u(t) * y{k-1}(t-1), y0 = z_pre
                # y1 covers t in [-2, SC), y2 [-1, SC), y3 [0, SC)
                y1 = wk.tile([P, SC + 2], F32, tag="y1")
                nc.vector.tensor_tensor(
                    y1[:], u_sb[:, : SC + 2], z_pre[:, HIST - 3 : HIST - 3 + SC + 2],
                    AL.mult,
                )
                y2 = wk.tile([P, SC + 1], F32, tag="y2")
                nc.vector.tensor_tensor(
                    y2[:], u_sb[:, 1 : SC + 2], y1[:, : SC + 1], AL.mult
                )
                y3 = wk.tile([P, SC], F32, tag="y3")
                nc.vector.tensor_tensor(
                    y3[:], u_sb[:, 2 : SC + 2], y2[:, :SC], AL.mult
                )
                # conv accumulation: acc = z*cw3 + y1*cw2 + y2*cw1 + y3*cw0
                acc = wk.tile([P, SC], F32, tag="acc")
                nc.vector.tensor_scalar(
                    acc[:], z_pre[:, HIST:SCH], cw_sb[:, 3:4], None, AL.mult
                )
                nc.vector.scalar_tensor_tensor(
                    acc[:], y1[:, 2 : SC + 2], cw_sb[:, 2:3], acc[:], AL.mult, AL.add
                )
                nc.vector.scalar_tensor_tensor(
                    acc[:], y2[:, 1 : SC + 1], cw_sb[:, 1:2], acc[:], AL.mult, AL.add
                )
                nc.vector.scalar_tensor_tensor(
                    acc[:], y3[:, :SC], cw_sb[:, 0:1], acc[:], AL.mult, AL.add
                )

                z_t = wk.tile([P, SC], F32, tag="y3")
                nc.scalar.activation(z_t[:], acc[:], ACTF.Sigmoid)
                na = wk.tile([P, SC], F32, tag="acc")
                nc.scalar.activation(na[:], acc[:], ACTF.Sigmoid, scale=-1.0)

                a_t = wk.tile([P, SC], F32, tag="a")
                nc.vector.tensor_tensor(a_t[:], na[:], u_sb[:, 2 : SC + 2], AL.mult)
                b_t = wk.tile([P, SC], F32, tag="b")
                for (no, nn, ps) in ps_h:
                    nc.vector.tensor_tensor(
                        b_t[:, no : no + nn], z_t[:, no : no + nn], ps[:, :nn], AL.mult
                    )

                h_loc = wk.tile([P, SC], F32, tag="hl")
                nc.vector.tensor_tensor_scan(
                    h_loc[:], a_t[:], b_t[:], 0.0, AL.mult, AL.add
                )
                A_t = wk.tile([P, SC], F32, tag="A")
                nc.vector.tensor_tensor_scan(
                    A_t[:], a_t[:], ones[:], 1.0, AL.mult, AL.mult
                )

                nc.scalar.copy(summA[:, m : m + 1], A_t[:, SC - 1 : SC])
                nc.scalar.copy(summH[:, m : m + 1], h_loc[:, SC - 1 : SC])
                nc.sync.dma_start(hl_d[m], h_loc[:])
                nc.sync.dma_start(A_d[m], A_t[:])
                if dbg_out is not None:
                    nc.sync.dma_start(dbg_out[0, m], h_loc[:])
                    nc.sync.dma_start(dbg_out[1, m], A_t[:])

            scopeA.__exit__(None, None, None)
            # ---- phase B: carry exchange ---------------------------------------
            mycarry = None
            if "B" in phases:
                nc.sync.dma_start(summ_d[:, 0:MT], summA[:])
                nc.sync.dma_start(summ_d[:, MT : 2 * MT], summH[:])
                nc.gpsimd.collective_compute(
                    "AllGather",
                    AL.bypass,
                    replica_groups=[list(range(NC))],
                    ins=[summ_d.opt()],
                    outs=[gath_d.opt()],
                )
                gsum = []
                for r in range(NC):
                    gs = cpool.tile([P, 2 * MT], F32, tag=f"gsum{r}", name=f"gsum{r}")
                    nc.sync.dma_start(gs[:], gath_d[r])
                    gsum.append(gs)
                state = cpool.tile([P, MT], F32, tag="cstate")
                tmp_c = cpool.tile([P, MT], F32, tag="ctmp")
                mycarry = cpool.tile([P, MT], F32, tag="mycarry")
                nc.any.memset(state[:], 0.0)
                nc.any.memset(mycarry[:], 0.0)
                for r in range(NC):
                    if r > 0:
                        # select carry entering rank r if this core is rank r
                        nc.vector.scalar_tensor_tensor(
                            mycarry[:], state[:], sel_sb[:, r : r + 1], mycarry[:],
                            AL.mult, AL.add,
                        )
                    if r < NC - 1:
                        nc.vector.tensor_tensor(
                            tmp_c[:], state[:], gsum[r][:, 0:MT], AL.mult
                        )
                        nc.vector.tensor_tensor(
                            state[:], tmp_c[:], gsum[r][:, MT : 2 * MT], AL.add
                        )
                if dbg_out is not None:
                    carry_dump = wk.tile([P, MT], F32, tag="cw")
                    nc.vector.tensor_copy(carry_dump[:], mycarry[:])
                    nc.sync.dma_start(dbg_out[2, 0, :, 0:MT], carry_dump[:])

            # ---- phase C: w0, silu, carry fixup, g -----------------------------
            if "C" in phases:
                scopeC = nc.named_scope("phaseC"); scopeC.__enter__()
                for m in range(MT):
                    w_sb = wpool.tile([P, KT, P], sbdt, tag="wz", name="w_sb")
                    nc.sync.dma_start(w_sb[:], w_in[m])
                    silu_t = wk.tile([P, SC], F32, tag="b")
                    for (no, nn) in s_nt:
                        ps = pp.tile([P, 512], F32, tag="ps", name="ps_w0")
                        for k in range(KT):
                            nc.tensor.matmul(
                                ps[:, :nn],
                                w_sb[:, k, :],
                                xt_sb[:, k, HIST + no : HIST + no + nn],
                                start=(k == 0),
                                stop=(k == KT - 1),
                            )
                        nc.scalar.activation(
                            silu_t[:, no : no + nn], ps[:, :nn], ACTF.Silu
                        )

                    hl_rd = wk.tile([P, SC], F32, tag="hl")
                    nc.sync.dma_start(hl_rd[:], hl_d[m])
                    A_rd = wk.tile([P, SC], F32, tag="A")
                    nc.sync.dma_start(A_rd[:], A_d[m])
                    h_true = wk.tile([P, SC], F32, tag="a")
                    nc.vector.scalar_tensor_tensor(
                        h_true[:], A_rd[:], mycarry[:, m : m + 1], hl_rd[:],
                        AL.mult, AL.add,
                    )
                    g_t = wk.tile([P, SC], MBF16, tag="g")
                    nc.vector.tensor_tensor(g_t[:], h_true[:], silu_t[:], AL.mult)
                    nc.sync.dma_start(g_d[m], g_t[:])
                    if dbg_out is not None and "D" not in phases:
                        if "S" in phases:
                            nc.sync.dma_start(dbg_out[2, m], silu_t[:])
                        else:
                            nc.sync.dma_start(dbg_out[2, m], h_true[:])

                scopeC.__exit__(None, None, None)

            # ---- phase D: down-projection out = g.T @ woT ----------------------
            if "D" in phases:
                scopeD = nc.named_scope("phaseD"); scopeD.__enter__()
                mp_all = list(range(MPT))
                for mbi in range(0, MPT, a_blk):
                    mps = mp_all[mbi : mbi + a_blk]
                    for nbi in range(0, len(NPT_D), b_blk):
                        nps = NPT_D[nbi : nbi + b_blk]
                        nb_off = nps[0][0]
                        nb_len = sum(nn for (_, nn) in nps)
                        ps_o = [
                            pp.tile([P, 512], F32, tag="ps", name=f"pso{i}")
                            for i in range(len(mps) * len(nps))
                        ]
                        for m in range(MT):
                            g_rd = wk.tile([P, P * len(mps)], MBF16, tag="y2")
                            nc.sync.dma_start(
                                g_rd[:], g_d[m][:, mbi * P : (mbi + len(mps)) * P]
                            )
                            wo_rd = wk.tile([P, nb_len], MBF16, tag="y1")
                            nc.sync.dma_start(
                                wo_rd[:], wo_in[m][:, nb_off : nb_off + nb_len]
                            )
                            for i_m in range(len(mps)):
                                for i_n, (no, nn) in enumerate(nps):
                                    nc.tensor.matmul(
                                        ps_o[i_m * len(nps) + i_n][:, :nn],
                                        g_rd[:, i_m * P : (i_m + 1) * P],
                                        wo_rd[:, no - nb_off : no - nb_off + nn],
                                        start=(m == 0),
                                        stop=(m == MT - 1),
                                    )
                        for i_m, mp in enumerate(mps):
                            o_sb = wk.tile([P, nb_len], F32, tag="zpre")
                            for i_n, (no, nn) in enumerate(nps):
                                nc.scalar.copy(
                                    o_sb[:, no - nb_off : no - nb_off + nn],
                                    ps_o[i_m * len(nps) + i_n][:, :nn],
                                )
                            nc.sync.dma_start(
                                out_d[mp * P : (mp + 1) * P, nb_off : nb_off + nb_len],
                                o_sb[:],
                            )
                scopeD.__exit__(None, None, None)
    nc.compile()
    return nc


def _prep_inputs(x, cu_seqlens, w_w, wz_w, wh_w, wo_w, conv_w, NC):
    """Host-side sharding + layout prep. Returns in_maps list."""
    S, D = x.shape[1], x.shape[2]
    H = w_w.shape[0]
    SC = S // NC
    KT, MT = D // P, H // P

    xT = np.ascontiguousarray(x[0].T.astype(np.float32))  # (D, S)
    xt_full = np.zeros((D, S + HIST), np.float32)
    xt_full[:, HIST:] = xT

    start = np.zeros(S, np.float32)
    for v in np.asarray(cu_seqlens[:-1]):
        v = int(v)
        if 0 <= v < S:
            start[v] = 1.0
    u = 1.0 - start
    u_full = np.ones(S + 2, np.float32)
    u_full[2:] = u  # index t+2 <-> position t

    mmnp = BF16 if MM_BF16 else np.float32

    def wprep(wm):  # (H, D) -> (MT, P, KT, P) with [m,p,k,j] = w[m*P+j, k*P+p]
        return np.ascontiguousarray(
            wm.astype(np.float32).reshape(MT, P, KT, P).transpose(0, 3, 2, 1)
        ).astype(mmnp)

    wz_t, wh_t, w_t = wprep(wz_w), wprep(wh_w), wprep(w_w)
    wo_t = np.ascontiguousarray(wo_w.T.astype(np.float32).reshape(MT, P, D)).astype(
        BF16
    )
    cw_t = np.ascontiguousarray(conv_w.astype(np.float32).reshape(MT, P, CONV))

    in_maps = []
    for c in range(NC):
        s0 = c * SC
        xt_c = np.ascontiguousarray(
            xt_full[:, s0 : s0 + SC + HIST]
            .reshape(KT, P, SC + HIST)
            .transpose(1, 0, 2)
        ).astype(mmnp)
        u_c = np.ascontiguousarray(
            np.broadcast_to(u_full[s0 : s0 + SC + 2], (P, SC + 2))
        )
        sel_c = np.zeros((P, NC), np.float32)
        sel_c[:, c] = 1.0
        in_maps.append(
            {
                "xt": xt_c,
                "wz": wz_t,
                "wh": wh_t,
                "w": w_t,
                "wo": wo_t,
                "cw": cw_t,
                "u": u_c,
                "sel": sel_c,
            }
        )
    return in_maps


_NC_CACHE = {}


def run_gru(x, cu_seqlens, w_w, wz_w, wh_w, wo_w, conv_w, NC=8, trace=False,
            phases="ABCD"):
    S, D = x.shape[1], x.shape[2]
    H = w_w.shape[0]
    SC = S // NC
    key = (D, H, SC, NC, phases)
    if key not in _NC_CACHE:
        _NC_CACHE[key] = build_gru_kernel(D, H, SC, NC, phases)
    nc = _NC_CACHE[key]
    in_maps = _prep_inputs(x, cu_seqlens, w_w, wz_w, wh_w, wo_w, conv_w, NC)
    res = run_bass_kernel_spmd(nc, in_maps, list(range(NC)), trace=trace)
    out = np.concatenate([res.results[c]["out"] for c in range(NC)], axis=0)
    return out.reshape(1, S, D).astype(np.float32), res


def kernel(**inputs):
    out, _ = run_gru(
        inputs["x"],
        inputs["cu_seqlens"],
        inputs["w_w"],
        inputs["wz_w"],
        inputs["wh_w"],
        inputs["wo_w"],
        inputs["conv_w"],
        NC=8,
    )
    return out



# revision 2
# speedup vs baseline: 1.1465x; 1.1465x over previous
"""Trainium2 Bass kernel for nn_GRU_15461882266204 (minGRU with causal conv gate).

Math (reference):
  w0 = x @ w_w.T ; z0 = x @ wz_w.T ; th = x @ wh_w.T          (S,H)
  z  = sigmoid(causal_conv4(z0, conv_w, segment-masked))
  a  = (1-z) * (1-start) ; b = z * th
  h_t = a_t * h_{t-1} + b_t                                    (scan over S)
  out = (h * silu(w0)) @ wo_w.T                                (S,D)

Strategy (v2): sequence-parallel over 8 NeuronCores (1024 positions each, all
5632 channels per core).  One fused phase A computes, per 128-channel m-tile:
the z / th / w0 projections (k-outer matmul groups sharing the xt moving
operand), the masked causal conv + sigmoid gates, the two hardware
tensor_tensor_scans (h_local, cumprod A), silu, and the bf16 products
g_loc = h_local*silu, gA = A*silu which stay SBUF-resident.  Cross-core scan
carry is exchanged in 11 chunked AllGathers of (A_end, h_end) summaries (4
m-tiles each) issued on the gpsimd queue as soon as each chunk's scans finish,
so collective latency overlaps the next chunk's matmuls.  The carry fixup
g = g_loc + carry*gA lands during phase A and writes final bf16 g tiles to
DRAM.  Phase D then runs 4 column passes of down-projection matmuls reading
pre-fixed g (2KB rows) + per-pass wo slices, sequence-sharded output (host
concatenates; no all-reduce).
"""
import sys

sys.path.insert(0, "/opt/trn_rl_repo")

import numpy as np

import concourse.bacc as bacc
import concourse.mybir as mybir
import concourse.tile as tile
from concourse.bass_utils import run_bass_kernel_spmd

try:
    import ml_dtypes

    BF16 = np.dtype(ml_dtypes.bfloat16)
except ImportError:  # pragma: no cover
    BF16 = None

F32 = mybir.dt.float32
MBF16 = mybir.dt.bfloat16
AL = mybir.AluOpType
ACTF = mybir.ActivationFunctionType

P = 128
CONV = 4
# 3 history columns are needed for the conv taps; pad to 4 (one dead leading
# column) to keep everything 4-aligned.
HIST = 4
CM = 4  # m-tiles per carry-exchange chunk


def build_gru_kernel(D, H, SC, NC):
    """Build the SPMD per-core program. SC = sequence chunk per core."""
    KT = D // P    # contraction k-tiles
    MT = H // P    # hidden m-tiles
    SCH = SC + HIST
    MPT = SC // P              # output row tiles (s on partitions)
    NB = D // 512              # down-proj column passes
    NCH = (MT + CM - 1) // CM  # carry chunks

    nc = bacc.Bacc(None, target_bir_lowering=False, debug=False)

    xt_in = nc.declare_dram_parameter("xt", [P, KT, SCH], MBF16, isOutput=False)
    wz_in = nc.declare_dram_parameter("wz", [MT, P, KT, P], MBF16, isOutput=False)
    wh_in = nc.declare_dram_parameter("wh", [MT, P, KT, P], MBF16, isOutput=False)
    w_in = nc.declare_dram_parameter("w", [MT, P, KT, P], MBF16, isOutput=False)
    wo_in = nc.declare_dram_parameter("wo", [NB, MT, P, 512], MBF16, isOutput=False)
    cw_in = nc.declare_dram_parameter("cw", [MT, P, CONV], F32, isOutput=False)
    u_in = nc.declare_dram_parameter("u", [P, SC + 2], MBF16, isOutput=False)
    sel_in = nc.declare_dram_parameter("sel", [P, NC], F32, isOutput=False)
    out_d = nc.declare_dram_parameter("out", [SC, D], F32, isOutput=True)

    with tile.TileContext(nc) as tc:
        with (
            tc.tile_pool(name="const", bufs=1) as cpool,
            tc.tile_pool(name="wts", bufs=2) as wpool,
            tc.tile_pool(name="work", bufs=2) as wk,
            tc.tile_pool(name="psum", bufs=8, space="PSUM") as pp,
            tc.tile_pool(name="dram", bufs=1, space="DRAM") as dp,
        ):
            # ---- resident tiles -------------------------------------------------
            # xt staged in 4 k-chunks so the first matmuls start early
            XCH = 4
            xt_sb = []
            for j in range(XCH):
                xj = cpool.tile([P, KT // XCH, SCH], MBF16, tag=f"xt{j}",
                                name=f"xt{j}")
                nc.sync.dma_start(xj[:], xt_in[:, j * (KT // XCH):(j + 1) * (KT // XCH), :])
                xt_sb.append(xj)

            def xt_slice(k, lo, hi):
                j = k // (KT // XCH)
                return xt_sb[j][:, k % (KT // XCH), lo:hi]

            u_sb = cpool.tile([P, SC + 2], MBF16, tag="u")
            nc.sync.dma_start(u_sb[:], u_in[:])
            sel_sb = cpool.tile([P, NC], F32, tag="sel")
            nc.sync.dma_start(sel_sb[:], sel_in[:])
            ones = cpool.tile([P, SC], MBF16, tag="ones")
            nc.any.memset(ones[:], 1.0)
            summA = cpool.tile([P, MT], F32, tag="summA")
            summH = cpool.tile([P, MT], F32, tag="summH")
            carry = cpool.tile([P, MT], F32, tag="carry")
            nc.any.memset(carry[:], 0.0)

            # internal DRAM buffers
            g_d = dp.tile([MT, P, SC], MBF16)
            summ_d = [dp.tile([P, 2 * CM], F32, tag=f"summ{c}", name=f"summ{c}")
                      for c in range(NCH)]
            gath_d = [dp.tile([NC, P, 2 * CM], F32, addr_space="Shared",
                              tag=f"gath{c}", name=f"gath{c}")
                      for c in range(NCH)]

            gl_tiles = {}
            gA_tiles = {}

            scopeA = nc.named_scope("phaseA"); scopeA.__enter__()
            for m in range(MT):
                cw_sb = wk.tile([P, CONV], F32, tag="cw")
                nc.sync.dma_start(cw_sb[:], cw_in[m])
                wz_sb = wpool.tile([P, KT, P], MBF16, tag="wz")
                nc.sync.dma_start(wz_sb[:], wz_in[m])
                wh_sb = wpool.tile([P, KT, P], MBF16, tag="wh")
                nc.sync.dma_start(wh_sb[:], wh_in[m])
                w_sb = wpool.tile([P, KT, P], MBF16, tag="w")
                nc.sync.dma_start(w_sb[:], w_in[m])

                # z_pre = wz_m.T @ x over SC+4 cols (history included), k-outer
                ps_z = [pp.tile([P, 512], F32, tag="ps", name=f"psz{i}")
                        for i in range(3)]
                for k in range(KT):
                    nc.tensor.matmul(ps_z[0][:, :512], wz_sb[:, k, :],
                                     xt_slice(k, 0, 512),
                                     start=(k == 0), stop=(k == KT - 1))
                    nc.tensor.matmul(ps_z[1][:, :512], wz_sb[:, k, :],
                                     xt_slice(k, 512, 1024),
                                     start=(k == 0), stop=(k == KT - 1))
                    nc.tensor.matmul(ps_z[2][:, :SCH - 1024], wz_sb[:, k, :],
                                     xt_slice(k, 1024, SCH),
                                     start=(k == 0), stop=(k == KT - 1))
                z_pre = wk.tile([P, SCH], MBF16, tag="zpre")
                nc.scalar.copy(z_pre[:, 0:512], ps_z[0][:, :512])
                nc.scalar.copy(z_pre[:, 512:1024], ps_z[1][:, :512])
                nc.scalar.copy(z_pre[:, 1024:SCH], ps_z[2][:, :SCH - 1024])

                # th matmuls (positions [0, SC) = cols [HIST, SCH))
                ps_h = [pp.tile([P, 512], F32, tag="ps", name=f"psh{i}")
                        for i in range(2)]
                for k in range(KT):
                    for i in range(2):
                        nc.tensor.matmul(
                            ps_h[i][:, :512], wh_sb[:, k, :],
                            xt_slice(k, HIST + i * 512, HIST + (i + 1) * 512),
                            start=(k == 0), stop=(k == KT - 1))
                th_sb = wk.tile([P, SC], MBF16, tag="th")
                nc.scalar.copy(th_sb[:, 0:512], ps_h[0][:, :512])
                nc.scalar.copy(th_sb[:, 512:1024], ps_h[1][:, :512])

                # w0 matmuls + silu
                ps_w = [pp.tile([P, 512], F32, tag="ps", name=f"psw{i}")
                        for i in range(2)]
                for k in range(KT):
                    for i in range(2):
                        nc.tensor.matmul(
                            ps_w[i][:, :512], w_sb[:, k, :],
                            xt_slice(k, HIST + i * 512, HIST + (i + 1) * 512),
                            start=(k == 0), stop=(k == KT - 1))
                silu_t = wk.tile([P, SC], MBF16, tag="silu")
                nc.scalar.activation(silu_t[:, 0:512], ps_w[0][:, :512], ACTF.Silu)
                nc.scalar.activation(silu_t[:, 512:1024], ps_w[1][:, :512], ACTF.Silu)

                # masked shifted taps: yk(t) = u(t) * y{k-1}(t-1), y0 = z_pre
                y1 = wk.tile([P, SC + 2], MBF16, tag="y1")
                nc.vector.tensor_tensor(
                    y1[:], u_sb[:, : SC + 2], z_pre[:, HIST - 3 : HIST - 3 + SC + 2],
                    AL.mult,
                )
                y2 = wk.tile([P, SC + 1], MBF16, tag="y2")
                nc.vector.tensor_tensor(
                    y2[:], u_sb[:, 1 : SC + 2], y1[:, : SC + 1], AL.mult
                )
                y3 = wk.tile([P, SC], MBF16, tag="y3")
                nc.vector.tensor_tensor(
                    y3[:], u_sb[:, 2 : SC + 2], y2[:, :SC], AL.mult
                )
                # conv accumulation: acc = z*cw3 + y1*cw2 + y2*cw1 + y3*cw0
                acc = wk.tile([P, SC], F32, tag="acc")
                nc.vector.tensor_scalar(
                    acc[:], z_pre[:, HIST:SCH], cw_sb[:, 3:4], None, AL.mult
                )
                nc.vector.scalar_tensor_tensor(
                    acc[:], y1[:, 2 : SC + 2], cw_sb[:, 2:3], acc[:], AL.mult, AL.add
                )
                nc.vector.scalar_tensor_tensor(
                    acc[:], y2[:, 1 : SC + 1], cw_sb[:, 1:2], acc[:], AL.mult, AL.add
                )
                nc.vector.scalar_tensor_tensor(
                    acc[:], y3[:, :SC], cw_sb[:, 0:1], acc[:], AL.mult, AL.add
                )

                z_t = wk.tile([P, SC], MBF16, tag="zt")
                nc.scalar.activation(z_t[:], acc[:], ACTF.Sigmoid)
                na = wk.tile([P, SC], MBF16, tag="na")
                nc.scalar.activation(na[:], acc[:], ACTF.Sigmoid, scale=-1.0)

                a_t = wk.tile([P, SC], MBF16, tag="a")
                nc.vector.tensor_tensor(a_t[:], na[:], u_sb[:, 2 : SC + 2], AL.mult)
                b_t = wk.tile([P, SC], MBF16, tag="b")
                nc.vector.tensor_tensor(b_t[:], z_t[:], th_sb[:], AL.mult)

                h_loc = wk.tile([P, SC], F32, tag="hl")
                nc.vector.tensor_tensor_scan(
                    h_loc[:], a_t[:], b_t[:], 0.0, AL.mult, AL.add
                )
                A_t = wk.tile([P, SC], F32, tag="A")
                nc.vector.tensor_tensor_scan(
                    A_t[:], a_t[:], ones[:], 1.0, AL.mult, AL.mult
                )

                nc.scalar.copy(summA[:, m : m + 1], A_t[:, SC - 1 : SC])
                nc.scalar.copy(summH[:, m : m + 1], h_loc[:, SC - 1 : SC])

                gl = wk.tile([P, SC], MBF16, tag="gl", bufs=2 * CM, name=f"gl{m}")
                nc.vector.tensor_tensor(gl[:], h_loc[:], silu_t[:], AL.mult)
                gA = wk.tile([P, SC], MBF16, tag="gA", bufs=2 * CM, name=f"gA{m}")
                nc.vector.tensor_tensor(gA[:], A_t[:], silu_t[:], AL.mult)
                gl_tiles[m] = gl
                gA_tiles[m] = gA

                # ---- chunk end: carry exchange + fixup --------------------------
                if (m + 1) % CM == 0 or m == MT - 1:
                    c = m // CM
                    m0 = c * CM
                    cm = m + 1 - m0
                    nc.gpsimd.dma_start(summ_d[c][:, 0:cm], summA[:, m0 : m0 + cm])
                    nc.gpsimd.dma_start(summ_d[c][:, CM : CM + cm],
                                        summH[:, m0 : m0 + cm])
                    nc.gpsimd.collective_compute(
                        "AllGather",
                        AL.bypass,
                        replica_groups=[list(range(NC))],
                        ins=[summ_d[c].opt()],
                        outs=[gath_d[c].opt()],
                    )
                    gsum = []
                    for r in range(NC):
                        gs = wk.tile([P, 2 * CM], F32, tag=f"gsum{r}",
                                     name=f"gsum{r}_{c}")
                        nc.gpsimd.dma_start(gs[:], gath_d[c][r])
                        gsum.append(gs)
                    state = wk.tile([P, CM], F32, tag="cstate")
                    tmp_c = wk.tile([P, CM], F32, tag="ctmp")
                    nc.vector.memset(state[:], 0.0)
                    for r in range(NC):
                        if r > 0:
                            nc.vector.scalar_tensor_tensor(
                                carry[:, m0 : m0 + cm], state[:, :cm],
                                sel_sb[:, r : r + 1], carry[:, m0 : m0 + cm],
                                AL.mult, AL.add,
                            )
                        if r < NC - 1:
                            nc.vector.tensor_tensor(
                                tmp_c[:, :cm], state[:, :cm], gsum[r][:, 0:cm],
                                AL.mult
                            )
                            nc.vector.tensor_tensor(
                                state[:, :cm], tmp_c[:, :cm],
                                gsum[r][:, CM : CM + cm], AL.add
                            )
                    for mm in range(m0, m0 + cm):
                        gfix = wk.tile([P, SC], MBF16, tag="gfix")
                        nc.vector.scalar_tensor_tensor(
                            gfix[:], gA_tiles[mm][:], carry[:, mm : mm + 1],
                            gl_tiles[mm][:], AL.mult, AL.add,
                        )
                        nc.gpsimd.dma_start(g_d[mm], gfix[:])
            scopeA.__exit__(None, None, None)

            # ---- phase D: down-projection out = g.T @ wo -----------------------
            scopeD = nc.named_scope("phaseD"); scopeD.__enter__()
            for nb in range(NB):
                ps_o = [pp.tile([P, 512], F32, tag="ps", name=f"pso{i}")
                        for i in range(MPT)]
                for m in range(MT):
                    g_rd = wk.tile([P, SC], MBF16, tag="g_rd", bufs=3)
                    nc.sync.dma_start(g_rd[:], g_d[m])
                    wo_rd = wk.tile([P, 512], MBF16, tag="wo_rd", bufs=3)
                    nc.sync.dma_start(wo_rd[:], wo_in[nb, m])
                    for mb in range(MPT):
                        nc.tensor.matmul(
                            ps_o[mb][:, :512],
                            g_rd[:, mb * P : (mb + 1) * P],
                            wo_rd[:],
                            start=(m == 0),
                            stop=(m == MT - 1),
                        )
                for mb in range(MPT):
                    o_sb = wk.tile([P, 512], F32, tag="o_sb", bufs=4)
                    nc.vector.tensor_copy(o_sb[:], ps_o[mb][:, :512])
                    nc.sync.dma_start(
                        out_d[mb * P : (mb + 1) * P, nb * 512 : (nb + 1) * 512],
                        o_sb[:],
                    )
            scopeD.__exit__(None, None, None)
    nc.compile()
    return nc


def _prep_inputs(x, cu_seqlens, w_w, wz_w, wh_w, wo_w, conv_w, NC):
    """Host-side sharding + layout prep. Returns in_maps list."""
    S, D = x.shape[1], x.shape[2]
    H = w_w.shape[0]
    SC = S // NC
    KT, MT = D // P, H // P
    NB = D // 512

    xT = np.ascontiguousarray(x[0].T.astype(np.float32))  # (D, S)
    xt_full = np.zeros((D, S + HIST), np.float32)
    xt_full[:, HIST:] = xT

    start = np.zeros(S, np.float32)
    for v in np.asarray(cu_seqlens[:-1]):
        v = int(v)
        if 0 <= v < S:
            start[v] = 1.0
    u = 1.0 - start
    u_full = np.ones(S + 2, np.float32)
    u_full[2:] = u  # index t+2 <-> position t

    def wprep(wm):  # (H, D) -> (MT, P, KT, P) with [m,p,k,j] = w[m*P+j, k*P+p]
        return np.ascontiguousarray(
            wm.astype(np.float32).reshape(MT, P, KT, P).transpose(0, 3, 2, 1)
        ).astype(BF16)

    wz_t, wh_t, w_t = wprep(wz_w), wprep(wh_w), wprep(w_w)
    # wo: [NB, MT, P, 512] with [nb,m,p,j] = wo[nb*512+j, m*128+p]
    wo_t = np.ascontiguousarray(
        wo_w.T.astype(np.float32).reshape(MT, P, NB, 512).transpose(2, 0, 1, 3)
    ).astype(BF16)
    cw_t = np.ascontiguousarray(conv_w.astype(np.float32).reshape(MT, P, CONV))

    in_maps = []
    for c in range(NC):
        s0 = c * SC
        xt_c = np.ascontiguousarray(
            xt_full[:, s0 : s0 + SC + HIST]
            .reshape(KT, P, SC + HIST)
            .transpose(1, 0, 2)
        ).astype(BF16)
        u_c = np.ascontiguousarray(
            np.broadcast_to(u_full[s0 : s0 + SC + 2], (P, SC + 2))
        ).astype(BF16)
        sel_c = np.zeros((P, NC), np.float32)
        sel_c[:, c] = 1.0
        in_maps.append(
            {
                "xt": xt_c,
                "wz": wz_t,
                "wh": wh_t,
                "w": w_t,
                "wo": wo_t,
                "cw": cw_t,
                "u": u_c,
                "sel": sel_c,
            }
        )
    return in_maps


_NC_CACHE = {}


def run_gru(x, cu_seqlens, w_w, wz_w, wh_w, wo_w, conv_w, NC=8, trace=False):
    S, D = x.shape[1], x.shape[2]
    H = w_w.shape[0]
    SC = S // NC
    key = (D, H, SC, NC)
    if key not in _NC_CACHE:
        _NC_CACHE[key] = build_gru_kernel(D, H, SC, NC)
    nc = _NC_CACHE[key]
    in_maps = _prep_inputs(x, cu_seqlens, w_w, wz_w, wh_w, wo_w, conv_w, NC)
    res = run_bass_kernel_spmd(nc, in_maps, list(range(NC)), trace=trace)
    out = np.concatenate([res.results[c]["out"] for c in range(NC)], axis=0)
    return out.reshape(1, S, D).astype(np.float32), res


def kernel(**inputs):
    out, _ = run_gru(
        inputs["x"],
        inputs["cu_seqlens"],
        inputs["w_w"],
        inputs["wz_w"],
        inputs["wh_w"],
        inputs["wo_w"],
        inputs["conv_w"],
        NC=8,
    )
    return out


# revision 14
# speedup vs baseline: 1.1756x; 1.0254x over previous
"""Trainium2 Bass kernel for nn_GRU_15461882266204 (minGRU with causal conv gate).

Math (reference):
  w0 = x @ w_w.T ; z0 = x @ wz_w.T ; th = x @ wh_w.T          (S,H)
  z  = sigmoid(causal_conv4(z0, conv_w, segment-masked))
  a  = (1-z) * (1-start) ; b = z * th
  h_t = a_t * h_{t-1} + b_t                                    (scan over S)
  out = (h * silu(w0)) @ wo_w.T                                (S,D)

Strategy (v2): sequence-parallel over 8 NeuronCores (1024 positions each, all
5632 channels per core).  One fused phase A computes, per 128-channel m-tile:
the z / th / w0 projections (k-outer matmul groups sharing the xt moving
operand), the masked causal conv + sigmoid gates, the two hardware
tensor_tensor_scans (h_local, cumprod A), silu, and the bf16 products
g_loc = h_local*silu, gA = A*silu which stay SBUF-resident.  Cross-core scan
carry is exchanged in 11 chunked AllGathers of (A_end, h_end) summaries (4
m-tiles each) issued on the gpsimd queue as soon as each chunk's scans finish,
so collective latency overlaps the next chunk's matmuls.  The carry fixup
g = g_loc + carry*gA lands during phase A and writes final bf16 g tiles to
DRAM.  Phase D then runs 4 column passes of down-projection matmuls reading
pre-fixed g (2KB rows) + per-pass wo slices, sequence-sharded output (host
concatenates; no all-reduce).
"""
import sys

sys.path.insert(0, "/opt/trn_rl_repo")

import numpy as np

import concourse.bacc as bacc
import concourse.mybir as mybir
import concourse.tile as tile
from concourse.bass_utils import run_bass_kernel_spmd

try:
    import ml_dtypes

    BF16 = np.dtype(ml_dtypes.bfloat16)
except ImportError:  # pragma: no cover
    BF16 = None

F32 = mybir.dt.float32
MBF16 = mybir.dt.bfloat16
AL = mybir.AluOpType
ACTF = mybir.ActivationFunctionType

P = 128
CONV = 4
# 3 history columns are needed for the conv taps; pad to 4 (one dead leading
# column) to keep everything 4-aligned.
HIST = 4
CM = 4  # m-tiles per carry-exchange chunk


def build_gru_kernel(D, H, SC, NC):
    """Build the SPMD per-core program. SC = sequence chunk per core."""
    KT = D // P    # contraction k-tiles
    MT = H // P    # hidden m-tiles
    SCH = SC + HIST
    MPT = SC // P              # output row tiles (s on partitions)
    NB = D // 512              # down-proj column passes
    NCH = (MT + CM - 1) // CM  # carry chunks

    nc = bacc.Bacc(None, target_bir_lowering=False, debug=False)

    xt_in = nc.declare_dram_parameter("xt", [P, KT, SCH], MBF16, isOutput=False)
    wz_in = nc.declare_dram_parameter("wz", [MT, P, KT, P], MBF16, isOutput=False)
    wh_in = nc.declare_dram_parameter("wh", [MT, P, KT, P], MBF16, isOutput=False)
    w_in = nc.declare_dram_parameter("w", [MT, P, KT, P], MBF16, isOutput=False)
    wo_in = nc.declare_dram_parameter("wo", [NB, MT, P, 512], MBF16, isOutput=False)
    cw_in = nc.declare_dram_parameter("cw", [MT, P, CONV], F32, isOutput=False)
    u_in = nc.declare_dram_parameter("u", [P, SC + 2], MBF16, isOutput=False)
    sel_in = nc.declare_dram_parameter("sel", [P, NC], F32, isOutput=False)
    out_d = nc.declare_dram_parameter("out", [SC, D], F32, isOutput=True)

    with tile.TileContext(nc) as tc:
        with (
            tc.tile_pool(name="const", bufs=1) as cpool,
            tc.tile_pool(name="wts", bufs=3) as wpool,
            tc.tile_pool(name="work", bufs=2) as wk,
            tc.tile_pool(name="psum", bufs=8, space="PSUM") as pp,
            tc.tile_pool(name="dram", bufs=1, space="DRAM") as dp,
        ):
            # ---- resident tiles -------------------------------------------------
            # Ramp-critical ordering: the very first matmul needs wz[0] and xt
            # chunk 0 — issue those DMAs first so the fair-share DMA engines
            # complete them before the bulk (xt tail, u, sel) transfers.
            XCH = 4
            wz0_sb = wpool.tile([P, KT, P], MBF16, tag="wz", name="wz0_sb")
            nc.sync.dma_start(wz0_sb[:], wz_in[0])
            xt_sb = []
            xj = cpool.tile([P, KT // XCH, SCH], MBF16, tag="xt0", name="xt0")
            nc.sync.dma_start(xj[:], xt_in[:, 0:KT // XCH, :])
            xt_sb.append(xj)
            wh0_sb = wpool.tile([P, KT, P], MBF16, tag="wh", name="wh0_sb")
            nc.sync.dma_start(wh0_sb[:], wh_in[0])
            for j in range(1, XCH):
                xj = cpool.tile([P, KT // XCH, SCH], MBF16, tag=f"xt{j}",
                                name=f"xt{j}")
                nc.sync.dma_start(xj[:], xt_in[:, j * (KT // XCH):(j + 1) * (KT // XCH), :])
                xt_sb.append(xj)
            w0_sb = wpool.tile([P, KT, P], MBF16, tag="w", name="w0_sb")
            nc.sync.dma_start(w0_sb[:], w_in[0])
            cw0_sb = wk.tile([P, CONV], F32, tag="cw", name="cw0_sb")
            nc.sync.dma_start(cw0_sb[:], cw_in[0])

            def xt_slice(k, lo, hi):
                j = k // (KT // XCH)
                return xt_sb[j][:, k % (KT // XCH), lo:hi]

            u_sb = cpool.tile([P, SC + 2], MBF16, tag="u")
            nc.sync.dma_start(u_sb[:], u_in[:])
            sel_sb = cpool.tile([P, NC], F32, tag="sel")
            nc.sync.dma_start(sel_sb[:], sel_in[:])
            ones = cpool.tile([P, SC], MBF16, tag="ones")
            nc.any.memset(ones[:], 1.0)
            summA = cpool.tile([P, MT], F32, tag="summA")
            summH = cpool.tile([P, MT], F32, tag="summH")
            carry = cpool.tile([P, MT], F32, tag="carry")
            nc.vector.memset(carry[:], 0.0)

            # internal DRAM buffers
            g_d = dp.tile([MT, P, SC], MBF16)
            summ_d = [dp.tile([P, 2 * CM], F32, tag=f"summ{c}", name=f"summ{c}")
                      for c in range(NCH)]
            gath_d = [dp.tile([NC, P, 2 * CM], F32, addr_space="Shared",
                              tag=f"gath{c}", name=f"gath{c}")
                      for c in range(NCH)]

            # Warmup collective: absorb the first-rendezvous latency across
            # cores while the first m-tiles compute.
            warm_d = dp.tile([P, 2 * CM], F32, tag="warm", name="warm_d")
            warmg_d = dp.tile([NC, P, 2 * CM], F32, addr_space="Shared",
                              tag="warmg", name="warmg_d")
            warm_sb = cpool.tile([P, 2 * CM], F32, tag="warm_sb")
            nc.vector.memset(warm_sb[:], 0.0)
            nc.gpsimd.dma_start(warm_d[:], warm_sb[:])
            nc.gpsimd.collective_compute(
                "AllGather",
                AL.bypass,
                replica_groups=[list(range(NC))],
                ins=[warm_d.opt()],
                outs=[warmg_d.opt()],
            )

            gl_tiles = {}
            gA_tiles = {}
            pending = []  # chunks whose carry chain/fixup is deferred

            def process_chunk(c, m0, cm, gsum):
                """Emit carry chain + fixup for chunk c (gath already read)."""
                state = wk.tile([P, CM], F32, tag="cstate", name=f"cst{c}")
                tmp_c = wk.tile([P, CM], F32, tag="ctmp", name=f"ctm{c}")
                nc.vector.memset(state[:], 0.0)
                for r in range(NC):
                    if r > 0:
                        nc.vector.scalar_tensor_tensor(
                            carry[:, m0 : m0 + cm], state[:, :cm],
                            sel_sb[:, r : r + 1], carry[:, m0 : m0 + cm],
                            AL.mult, AL.add,
                        )
                    if r < NC - 1:
                        nc.vector.tensor_tensor(
                            tmp_c[:, :cm], state[:, :cm], gsum[r][:, 0:cm],
                            AL.mult
                        )
                        nc.vector.tensor_tensor(
                            state[:, :cm], tmp_c[:, :cm],
                            gsum[r][:, CM : CM + cm], AL.add
                        )
                for mm in range(m0, m0 + cm):
                    gfix = wk.tile([P, SC], MBF16, tag="gfix", name=f"gfx{mm}")
                    nc.vector.scalar_tensor_tensor(
                        gfix[:], gA_tiles[mm][:], carry[:, mm : mm + 1],
                        gl_tiles[mm][:], AL.mult, AL.add,
                    )
                    nc.gpsimd.dma_start(g_d[mm], gfix[:])
                    del gl_tiles[mm], gA_tiles[mm]

            scopeA = nc.named_scope("phaseA"); scopeA.__enter__()
            for m in range(MT):
                # flush deferred carry chunks once the collective has had ~2
                # m-tiles (~50us) to complete — keeps the DVE FIFO from
                # head-blocking on the gather.
                while pending and m >= pending[0][1] + pending[0][2] + 2:
                    process_chunk(*pending.pop(0))
                if m == 0:
                    cw_sb, wz_sb, wh_sb, w_sb = cw0_sb, wz0_sb, wh0_sb, w0_sb
                else:
                    cw_sb = wk.tile([P, CONV], F32, tag="cw")
                    nc.sync.dma_start(cw_sb[:], cw_in[m])
                    wz_sb = wpool.tile([P, KT, P], MBF16, tag="wz")
                    nc.sync.dma_start(wz_sb[:], wz_in[m])
                    wh_sb = wpool.tile([P, KT, P], MBF16, tag="wh")
                    nc.sync.dma_start(wh_sb[:], wh_in[m])
                    w_sb = wpool.tile([P, KT, P], MBF16, tag="w")
                    nc.sync.dma_start(w_sb[:], w_in[m])

                # z_pre = wz_m.T @ x over SC+4 cols (history included), k-outer
                ps_z = [pp.tile([P, 512], F32, tag="ps", name=f"psz{i}")
                        for i in range(3)]
                for k in range(KT):
                    nc.tensor.matmul(ps_z[0][:, :512], wz_sb[:, k, :],
                                     xt_slice(k, 0, 512),
                                     start=(k == 0), stop=(k == KT - 1))
                    nc.tensor.matmul(ps_z[1][:, :512], wz_sb[:, k, :],
                                     xt_slice(k, 512, 1024),
                                     start=(k == 0), stop=(k == KT - 1))
                    nc.tensor.matmul(ps_z[2][:, :SCH - 1024], wz_sb[:, k, :],
                                     xt_slice(k, 1024, SCH),
                                     start=(k == 0), stop=(k == KT - 1))
                z_pre = wk.tile([P, SCH], MBF16, tag="zpre")
                nc.scalar.copy(z_pre[:, 0:512], ps_z[0][:, :512])
                nc.scalar.copy(z_pre[:, 512:1024], ps_z[1][:, :512])
                nc.scalar.copy(z_pre[:, 1024:SCH], ps_z[2][:, :SCH - 1024])

                # th matmuls (positions [0, SC) = cols [HIST, SCH))
                ps_h = [pp.tile([P, 512], F32, tag="ps", name=f"psh{i}")
                        for i in range(2)]
                for k in range(KT):
                    for i in range(2):
                        nc.tensor.matmul(
                            ps_h[i][:, :512], wh_sb[:, k, :],
                            xt_slice(k, HIST + i * 512, HIST + (i + 1) * 512),
                            start=(k == 0), stop=(k == KT - 1))
                th_sb = wk.tile([P, SC], MBF16, tag="th", bufs=3)
                nc.scalar.copy(th_sb[:, 0:512], ps_h[0][:, :512])
                nc.scalar.copy(th_sb[:, 512:1024], ps_h[1][:, :512])

                # w0 matmuls + silu
                ps_w = [pp.tile([P, 512], F32, tag="ps", name=f"psw{i}")
                        for i in range(2)]
                for k in range(KT):
                    for i in range(2):
                        nc.tensor.matmul(
                            ps_w[i][:, :512], w_sb[:, k, :],
                            xt_slice(k, HIST + i * 512, HIST + (i + 1) * 512),
                            start=(k == 0), stop=(k == KT - 1))
                silu_t = wk.tile([P, SC], MBF16, tag="silu")
                nc.scalar.activation(silu_t[:, 0:512], ps_w[0][:, :512], ACTF.Silu)
                nc.scalar.activation(silu_t[:, 512:1024], ps_w[1][:, :512], ACTF.Silu)

                # masked shifted taps: yk(t) = u(t) * y{k-1}(t-1), y0 = z_pre
                y1 = wk.tile([P, SC + 2], MBF16, tag="y1")
                nc.vector.tensor_tensor(
                    y1[:], u_sb[:, : SC + 2], z_pre[:, HIST - 3 : HIST - 3 + SC + 2],
                    AL.mult,
                )
                y2 = wk.tile([P, SC + 1], MBF16, tag="y2")
                nc.vector.tensor_tensor(
                    y2[:], u_sb[:, 1 : SC + 2], y1[:, : SC + 1], AL.mult
                )
                y3 = wk.tile([P, SC], MBF16, tag="y3")
                nc.vector.tensor_tensor(
                    y3[:], u_sb[:, 2 : SC + 2], y2[:, :SC], AL.mult
                )
                # conv accumulation: acc = z*cw3 + y1*cw2 + y2*cw1 + y3*cw0
                acc = wk.tile([P, SC], F32, tag="acc")
                nc.vector.tensor_scalar(
                    acc[:], z_pre[:, HIST:SCH], cw_sb[:, 3:4], None, AL.mult
                )
                nc.vector.scalar_tensor_tensor(
                    acc[:], y1[:, 2 : SC + 2], cw_sb[:, 2:3], acc[:], AL.mult, AL.add
                )
                nc.vector.scalar_tensor_tensor(
                    acc[:], y2[:, 1 : SC + 1], cw_sb[:, 1:2], acc[:], AL.mult, AL.add
                )
                nc.vector.scalar_tensor_tensor(
                    acc[:], y3[:, :SC], cw_sb[:, 0:1], acc[:], AL.mult, AL.add
                )

                z_t = wk.tile([P, SC], MBF16, tag="zt")
                nc.scalar.activation(z_t[:], acc[:], ACTF.Sigmoid)
                na = wk.tile([P, SC], MBF16, tag="na")
                nc.scalar.activation(na[:], acc[:], ACTF.Sigmoid, scale=-1.0)

                a_t = wk.tile([P, SC], MBF16, tag="a")
                nc.vector.tensor_tensor(a_t[:], na[:], u_sb[:, 2 : SC + 2], AL.mult)
                b_t = wk.tile([P, SC], MBF16, tag="b")
                nc.vector.tensor_tensor(b_t[:], z_t[:], th_sb[:], AL.mult)

                h_loc = wk.tile([P, SC], F32, tag="hl")
                nc.vector.tensor_tensor_scan(
                    h_loc[:], a_t[:], b_t[:], 0.0, AL.mult, AL.add
                )
                A_t = wk.tile([P, SC], F32, tag="A")
                nc.vector.tensor_tensor_scan(
                    A_t[:], a_t[:], ones[:], 1.0, AL.mult, AL.mult
                )

                nc.scalar.copy(summA[:, m : m + 1], A_t[:, SC - 1 : SC])
                nc.scalar.copy(summH[:, m : m + 1], h_loc[:, SC - 1 : SC])

                GBUFS = 2 * CM
                gl = wk.tile([P, SC], MBF16, tag="gl", bufs=GBUFS, name=f"gl{m}")
                nc.vector.tensor_tensor(gl[:], h_loc[:], silu_t[:], AL.mult)
                gA = wk.tile([P, SC], MBF16, tag="gA", bufs=GBUFS, name=f"gA{m}")
                nc.vector.tensor_tensor(gA[:], A_t[:], silu_t[:], AL.mult)
                gl_tiles[m] = gl
                gA_tiles[m] = gA

                # ---- chunk end: issue summary AllGather (carry deferred) --------
                if (m + 1) % CM == 0 or m == MT - 1:
                    c = m // CM
                    m0 = c * CM
                    cm = m + 1 - m0
                    nc.gpsimd.dma_start(summ_d[c][:, 0:cm], summA[:, m0 : m0 + cm])
                    nc.gpsimd.dma_start(summ_d[c][:, CM : CM + cm],
                                        summH[:, m0 : m0 + cm])
                    nc.gpsimd.collective_compute(
                        "AllGather",
                        AL.bypass,
                        replica_groups=[list(range(NC))],
                        ins=[summ_d[c].opt()],
                        outs=[gath_d[c].opt()],
                    )
                    gsum = []
                    for r in range(NC):
                        gs = wk.tile([P, 2 * CM], F32, tag=f"gsum{r}",
                                     name=f"gsum{r}_{c}")
                        nc.gpsimd.dma_start(gs[:], gath_d[c][r])
                        gsum.append(gs)
                    pending.append((c, m0, cm, gsum))
            while pending:
                process_chunk(*pending.pop(0))
            scopeA.__exit__(None, None, None)

            # ---- phase D: down-projection out = g.T @ wo -----------------------
            scopeD = nc.named_scope("phaseD"); scopeD.__enter__()
            for nb in range(NB):
                ps_o = [pp.tile([P, 512], F32, tag="ps", name=f"pso{i}")
                        for i in range(MPT)]
                for m in range(MT):
                    g_rd = wk.tile([P, SC], MBF16, tag="g_rd", bufs=4)
                    nc.sync.dma_start(g_rd[:], g_d[m])
                    wo_rd = wk.tile([P, 512], MBF16, tag="wo_rd", bufs=4)
                    nc.sync.dma_start(wo_rd[:], wo_in[nb, m])
                    for mb in range(MPT):
                        nc.tensor.matmul(
                            ps_o[mb][:, :512],
                            g_rd[:, mb * P : (mb + 1) * P],
                            wo_rd[:],
                            start=(m == 0),
                            stop=(m == MT - 1),
                        )
                for mb in range(MPT):
                    o_sb = wk.tile([P, 512], F32, tag="o_sb", bufs=8)
                    nc.vector.tensor_copy(o_sb[:], ps_o[mb][:, :512])
                    nc.sync.dma_start(
                        out_d[mb * P : (mb + 1) * P, nb * 512 : (nb + 1) * 512],
                        o_sb[:],
                    )
            scopeD.__exit__(None, None, None)
    nc.compile()
    return nc


def _prep_inputs(x, cu_seqlens, w_w, wz_w, wh_w, wo_w, conv_w, NC):
    """Host-side sharding + layout prep. Returns in_maps list."""
    S, D = x.shape[1], x.shape[2]
    H = w_w.shape[0]
    SC = S // NC
    KT, MT = D // P, H // P
    NB = D // 512

    xT = np.ascontiguousarray(x[0].T.astype(np.float32))  # (D, S)
    xt_full = np.zeros((D, S + HIST), np.float32)
    xt_full[:, HIST:] = xT

    start = np.zeros(S, np.float32)
    for v in np.asarray(cu_seqlens[:-1]):
        v = int(v)
        if 0 <= v < S:
            start[v] = 1.0
    u = 1.0 - start
    u_full = np.ones(S + 2, np.float32)
    u_full[2:] = u  # index t+2 <-> position t

    def wprep(wm):  # (H, D) -> (MT, P, KT, P) with [m,p,k,j] = w[m*P+j, k*P+p]
        return np.ascontiguousarray(
            wm.astype(np.float32).reshape(MT, P, KT, P).transpose(0, 3, 2, 1)
        ).astype(BF16)

    wz_t, wh_t, w_t = wprep(wz_w), wprep(wh_w), wprep(w_w)
    # wo: [NB, MT, P, 512] with [nb,m,p,j] = wo[nb*512+j, m*128+p]
    wo_t = np.ascontiguousarray(
        wo_w.T.astype(np.float32).reshape(MT, P, NB, 512).transpose(2, 0, 1, 3)
    ).astype(BF16)
    cw_t = np.ascontiguousarray(conv_w.astype(np.float32).reshape(MT, P, CONV))

    in_maps = []
    for c in range(NC):
        s0 = c * SC
        xt_c = np.ascontiguousarray(
            xt_full[:, s0 : s0 + SC + HIST]
            .reshape(KT, P, SC + HIST)
            .transpose(1, 0, 2)
        ).astype(BF16)
        u_c = np.ascontiguousarray(
            np.broadcast_to(u_full[s0 : s0 + SC + 2], (P, SC + 2))
        ).astype(BF16)
        sel_c = np.zeros((P, NC), np.float32)
        sel_c[:, c] = 1.0
        in_maps.append(
            {
                "xt": xt_c,
                "wz": wz_t,
                "wh": wh_t,
                "w": w_t,
                "wo": wo_t,
                "cw": cw_t,
                "u": u_c,
                "sel": sel_c,
            }
        )
    return in_maps


_NC_CACHE = {}


def run_gru(x, cu_seqlens, w_w, wz_w, wh_w, wo_w, conv_w, NC=8, trace=False):
    S, D = x.shape[1], x.shape[2]
    H = w_w.shape[0]
    SC = S // NC
    key = (D, H, SC, NC)
    if key not in _NC_CACHE:
        _NC_CACHE[key] = build_gru_kernel(D, H, SC, NC)
    nc = _NC_CACHE[key]
    in_maps = _prep_inputs(x, cu_seqlens, w_w, wz_w, wh_w, wo_w, conv_w, NC)
    res = run_bass_kernel_spmd(nc, in_maps, list(range(NC)), trace=trace)
    out = np.concatenate([res.results[c]["out"] for c in range(NC)], axis=0)
    return out.reshape(1, S, D).astype(np.float32), res


def kernel(**inputs):
    out, _ = run_gru(
        inputs["x"],
        inputs["cu_seqlens"],
        inputs["w_w"],
        inputs["wz_w"],
        inputs["wh_w"],
        inputs["wo_w"],
        inputs["conv_w"],
        NC=8,
    )
    return out
